# revision 25
# baseline (speedup 1.0000x reference)
"""Multi-head attention (B=2, S=2048, D=1024, H=16, d_head=64) on 8 TRN2 cores.

Sharding: 2-way data parallel over batch x 4-way tensor parallel over heads.
Core c: batch g = c//4, heads [4r, 4r+4) with r = c%4. Each core projects
Q/K/V for its 4 heads from its batch's (pre-transposed) activations, runs
attention per head in a transposed layout (scores^T with keys on partitions),
then AllGathers the per-core head outputs within each 4-core batch group and
computes a 256-row slice of the Wout projection (column parallel). The host
concatenates the per-core output slices.

Layout notes:
  - All matmul operands bf16; accumulation f32 in PSUM.
  - scores^T tiles [128 k, 2048 q] are written to PSUM as bf16 (2 banks,
    non-accumulating) so one ACT exp instruction covers a full k-tile.
  - softmax denominators ride as a 65th "ones" column of V in the PV matmul;
    normalization multiplies by the PE-broadcast reciprocal row.
"""

import os
import sys

import numpy as np

for _p in ("/opt/trn_rl_repo",):
    if _p not in sys.path and os.path.isdir(_p):
        sys.path.append(_p)

import ml_dtypes

import concourse.bacc as bacc
import concourse.bass_utils as _bu
import concourse.mybir as mybir
from concourse.bass_utils import run_bass_kernel_spmd
from concourse.tile import TileContext

# Let walrus dedup LDWEIGHTS for consecutive matmuls that share a stationary
# operand; without it every matmul reloads its weights and the reload gaps
# keep the PE clock throttled.
if not getattr(_bu, "_ldw_opt_patched", False):
    _orig_run_command = _bu.run_command

    def _run_command_ldw(cmd, *a, **kw):
        cmd = [
            c  # ldw-opt=true fails walrus codegen on this BIR; keep default
            if isinstance(c, str)
            else c
            for c in cmd
        ]
        return _orig_run_command(cmd, *a, **kw)

    _bu.run_command = _run_command_ldw
    _bu._ldw_opt_patched = True

P = 128
B, S, DM = 2, 2048, 1024
NH_TOT, EH = 16, 64  # total heads, head dim
NCORES = 8
GROUPS = 2  # batch groups of 4 cores
NH = 4  # heads per core
EHC = NH * EH  # 256: head-concat width per core
NDT = DM // P  # 8 d-tiles
NKT = S // P  # 16 key tiles
QC = 512  # q chunk
NQC = S // QC  # 4
VW = EH + 1  # V width incl. ones column

BF = mybir.dt.bfloat16
F32 = mybir.dt.float32
F32R = mybir.dt.float32r

_cached_nc = None


def build_nc():
    nc = bacc.Bacc("TRN2", target_bir_lowering=False, debug=False, num_devices=NCORES)

    xqt = nc.declare_dram_parameter("xqt", [DM, S], BF, isOutput=False)
    xkt = nc.declare_dram_parameter("xkt", [DM, S], BF, isOutput=False)
    xvt = nc.declare_dram_parameter("xvt", [DM, S], BF, isOutput=False)
    wqt = nc.declare_dram_parameter("wqt", [DM, EHC], BF, isOutput=False)
    wkt = nc.declare_dram_parameter("wkt", [DM, EHC], BF, isOutput=False)
    wvt = nc.declare_dram_parameter("wvt", [DM, EHC], BF, isOutput=False)
    wot = nc.declare_dram_parameter("wot", [DM, EHC], BF, isOutput=False)
    outt = nc.declare_dram_parameter("outt", [EHC, S], F32, isOutput=True)

    with TileContext(nc) as tc:
        with (
            tc.tile_pool(name="persist", bufs=1) as persist,
            tc.tile_pool(name="dram", bufs=1, space="DRAM") as dram,
        ):
            # --- persistent SBUF ---
            wq_sb = persist.tile([P, NDT, EHC], BF)
            wk_sb = persist.tile([P, NDT, EHC], BF)
            wv_sb = persist.tile([P, NDT, EHC], BF)
            wo_sb = persist.tile([P, NDT, EHC], BF)
            for wsb, wpar in ((wq_sb, wqt), (wk_sb, wkt), (wv_sb, wvt), (wo_sb, wot)):
                nc.sync.dma_start(wsb[:], wpar.rearrange("(dt p) e -> p dt e", p=P))

            qt_sb = [persist.tile([P, S], BF, name=f"qt{et}") for et in range(2)]
            kt_sb = [persist.tile([P, S], BF, name=f"kt{et}") for et in range(2)]
            v_sb = persist.tile([P, NKT * NH * VW + P - VW], BF)
            heads_sb = [persist.tile([EH, S], BF, name=f"hd{h}") for h in range(NH)]
            xv_sb = [persist.tile([P, S], BF, name=f"xv{dt}") for dt in range(NDT)]
            nc.gpsimd.memset(v_sb[:], 1.0)  # ones column; V data overwrites cols 0:64

            for dt in range(NDT):
                nc.sync.dma_start(xv_sb[dt][:], xvt[dt * P : (dt + 1) * P, :])

            heads_loc = [
                dram.tile([EHC, 1024], BF, name=f"hloc{qh}") for qh in range(2)
            ]
            heads_all = [
                dram.tile([4 * EHC, 1024], BF, name=f"hall_d{qh}") for qh in range(2)
            ]

            def emit_allgather(qh):
                nc.gpsimd.collective_compute(
                    "AllGather",
                    mybir.AluOpType.bypass,
                    replica_groups=[[0, 1, 2, 3], [4, 5, 6, 7]],
                    ins=[heads_loc[qh].opt()],
                    outs=[heads_all[qh].opt()],
                )

            # --- Q/K projections: Q^T/K^T = W^T.T @ x^T, e on partitions ---
            with (
                tc.tile_pool(name="xin", bufs=3) as xin,
                tc.tile_pool(name="projp", bufs=1, space="PSUM") as projp,
            ):
                for xpar, wsb, dst in ((xqt, wq_sb, qt_sb), (xkt, wk_sb, kt_sb)):
                    ps = [
                        [
                            projp.tile([P, QC], F32, name=f"pp{et}_{qc}")
                            for qc in range(NQC)
                        ]
                        for et in range(2)
                    ]
                    for dt in range(NDT):
                        xt = xin.tile([P, S], BF, name="xt", tag="xt")
                        nc.sync.dma_start(xt[:], xpar[dt * P : (dt + 1) * P, :])
                        for et in range(2):
                            for qc in range(NQC):
                                nc.tensor.matmul(
                                    ps[et][qc][:],
                                    wsb[:, dt, et * P : (et + 1) * P],
                                    xt[:, qc * QC : (qc + 1) * QC],
                                    start=(dt == 0),
                                    stop=(dt == NDT - 1),
                                )
                    for et in range(2):
                        for qc in range(NQC):
                            nc.vector.tensor_copy(
                                dst[et][:, qc * QC : (qc + 1) * QC], ps[et][qc][:]
                            )

            # --- V projection (token-major): V = x^T.T @ Wv^T, tok on partitions ---
            with tc.tile_pool(name="vp", bufs=2, space="PSUM") as vp:
                for tt in range(NKT):
                    psv = vp.tile([P, EHC], F32, name="psv")
                    for dt in range(NDT):
                        nc.tensor.matmul(
                            psv[:],
                            xv_sb[dt][:, tt * P : (tt + 1) * P],
                            wv_sb[:, dt, :],
                            start=(dt == 0),
                            stop=(dt == NDT - 1),
                        )
                    nc.vector.tensor_copy(
                        v_sb[:, tt * NH * VW : (tt + 1) * NH * VW].rearrange(
                            "p (h w) -> p h w", w=VW
                        )[:, :, 0:EH],
                        psv[:].rearrange("p (h e) -> p h e", e=EH),
                    )

            # --- attention: head pairs (e-tiles), row-tiled scores ---
            # The two heads of an e-tile compute scores concurrently via PE
            # row tiling (head A on array rows 0-63, head B on 64-127), and
            # PV uses a [128,128] stationary (V | ones | junk) so every
            # matmul drives the full array. q is processed in halves of 1024
            # so the 8 PSUM banks cover pair scores (4) + 4 PV accumulators.
            with (
                tc.tile_pool(name="scorep", bufs=1, space="PSUM") as scorep,
                tc.tile_pool(name="pvp", bufs=1, space="PSUM") as pvp,
                tc.tile_pool(name="exps", bufs=3) as expp,
                tc.tile_pool(name="normp", bufs=2) as normp,
            ):

                def normalize(h, qh, col0, pvt):
                    # heads[h][:, col0:col0+512] = pv[0:64] * bcast(1/pv[64]).
                    # The 1-lane denominator row is bounced through DRAM to
                    # spread it across 128 partitions for a fast reciprocal,
                    # then broadcast-read back across 64 partitions.
                    den = normp.tile([VW, QC], F32, name="den", tag="den")
                    nc.vector.tensor_copy(den[EH : EH + 1, :], pvt[EH : EH + 1, :])
                    den_d = dram.tile([QC], F32, name="den_d", tag="den_d", bufs=2)
                    nc.sync.dma_start(den_d[:], den[EH : EH + 1, :])
                    dsp = normp.tile([P, NQC], F32, name="dsp", tag="dsp")
                    nc.sync.dma_start(dsp[:], den_d[:].rearrange("(p f) -> p f", p=P))
                    rsp = normp.tile([P, NQC], F32, name="rsp", tag="rsp")
                    nc.vector.reciprocal(rsp[:], dsp[:])
                    rcp_d = dram.tile([QC], F32, name="rcp_d", tag="rcp_d", bufs=2)
                    nc.sync.dma_start(rcp_d[:].rearrange("(p f) -> p f", p=P), rsp[:])
                    bc = normp.tile([EH, QC], F32, name="bc", tag="bc")
                    nc.sync.dma_start(bc[:], rcp_d[None, :].to_broadcast([EH, QC]))
                    nc.vector.tensor_mul(
                        heads_sb[h][:, col0 : col0 + QC],
                        pvt[0:EH, :],
                        bc[:],
                    )
                    nc.sync.dma_start(
                        heads_loc[qh][
                            h * EH : (h + 1) * EH, col0 - qh * 1024 : col0 - qh * 1024 + QC
                        ],
                        heads_sb[h][:, col0 : col0 + QC],
                    )

                def voff(kt, h):
                    return (kt * NH + h) * VW

                # q-half outer so that after two sweeps every head has its
                # first 1024 q columns finished and the (expensive, mostly
                # fixed-cost) AllGather for that half launches early and
                # hides under the remaining sweeps.
                pending = []  # deferred (h, qh, col0, pv_tile) normalizations
                for qh in range(2):
                    q0 = qh * 1024
                    for ep in range(2):
                        hA, hB = 2 * ep, 2 * ep + 1
                        pv = [
                            [
                                pvp.tile(
                                    [P, QC], F32, name=f"pv{lh}{q2}", tag=f"pv{lh}{q2}"
                                )
                                for q2 in range(2)
                            ]
                            for lh in range(2)
                        ]
                        for kt in range(NKT):
                            expair = expp.tile([P, S], BF, name="expair", tag="expair")
                            spair = scorep.tile([P, S], F32, name="spair", tag="spair")
                            for sub in range(2):
                                qs = q0 + sub * QC
                                nc.tensor.matmul(
                                    spair[:, sub * QC : (sub + 1) * QC],
                                    kt_sb[ep][0:EH, kt * P : (kt + 1) * P],
                                    qt_sb[ep][0:EH, qs : qs + QC],
                                    start=True,
                                    stop=True,
                                )
                                nc.tensor.matmul(
                                    spair[:, 1024 + sub * QC : 1024 + (sub + 1) * QC],
                                    kt_sb[ep][EH:P, kt * P : (kt + 1) * P],
                                    qt_sb[ep][EH:P, qs : qs + QC],
                                    start=True,
                                    stop=True,
                                )
                            nc.scalar.activation(
                                expair[:],
                                spair[:],
                                mybir.ActivationFunctionType.Exp,
                                scale=float(1.0 / np.sqrt(EH)),
                            )
                            for lh in range(2):
                                h = hA if lh == 0 else hB
                                for q2 in range(2):
                                    nc.tensor.matmul(
                                        pv[lh][q2][:],
                                        v_sb[:, voff(kt, h) : voff(kt, h) + P],
                                        expair[
                                            :,
                                            lh * 1024 + q2 * QC : lh * 1024
                                            + (q2 + 1) * QC,
                                        ],
                                        start=(kt == 0),
                                        stop=(kt == NKT - 1),
                                        skip_group_check=True,
                                    )
                            if kt == 2 and pending:
                                flush_qh, flush_ep = pending[0][1], pending[0][2]
                                for ph, _, _, pcol0, ppv in pending:
                                    normalize(ph, flush_qh, pcol0, ppv)
                                pending = []
                                if flush_qh == 0 and flush_ep == 1:
                                    # all four heads' first q-half is done
                                    emit_allgather(0)
                        for lh in range(2):
                            h = hA if lh == 0 else hB
                            for q2 in range(2):
                                pending.append((h, qh, ep, q0 + q2 * QC, pv[lh][q2]))
                flush_qh = pending[0][1]
                for ph, _, _, pcol0, ppv in pending:
                    normalize(ph, flush_qh, pcol0, ppv)
                emit_allgather(1)

            # --- Wout (column-parallel slice): out^T = Wout_slice^T.T @ heads^T ---
            with (
                tc.tile_pool(name="hall", bufs=1) as hallp,
                tc.tile_pool(name="wop", bufs=2, space="PSUM") as wop,
                tc.tile_pool(name="outp", bufs=1) as outp,
            ):
                hall = [
                    [hallp.tile([P, 1024], BF, name=f"hall{qh}_{dt}") for dt in range(NDT)]
                    for qh in range(2)
                ]
                for qh in range(2):
                    for dt in range(NDT):
                        nc.sync.dma_start(
                            hall[qh][dt][:], heads_all[qh][dt * P : (dt + 1) * P, :]
                        )
                out_sb = [outp.tile([P, S], F32, name=f"ot{ot}") for ot in range(2)]
                for qh in range(2):
                    for ot in range(2):
                        for q2 in range(2):
                            pso = wop.tile([P, QC], F32, name="pso", tag="pso")
                            for dt in range(NDT):
                                nc.tensor.matmul(
                                    pso[:],
                                    wo_sb[:, dt, ot * P : (ot + 1) * P],
                                    hall[qh][dt][:, q2 * QC : (q2 + 1) * QC],
                                    start=(dt == 0),
                                    stop=(dt == NDT - 1),
                                )
                            nc.vector.tensor_copy(
                                out_sb[ot][:, qh * 1024 + q2 * QC : qh * 1024 + (q2 + 1) * QC],
                                pso[:],
                            )
                for ot in range(2):
                    nc.sync.dma_start(outt[ot * P : (ot + 1) * P, :], out_sb[ot][:])

    nc.compile()
    return nc


def _prep_inputs(x_query, x_key, x_value, Wq, Wk, Wv, Wout):
    bf = ml_dtypes.bfloat16
    xt = {}
    for g in range(GROUPS):
        xt[g] = tuple(
            np.ascontiguousarray(np.asarray(x[g], dtype=np.float32).T).astype(bf)
            for x in (x_query, x_key, x_value)
        )
    in_maps = []
    for c in range(NCORES):
        g, r = c // 4, c % 4
        hs = slice(NH * r, NH * (r + 1))
        wq_c = np.ascontiguousarray(
            np.asarray(Wq[hs], dtype=np.float32).reshape(EHC, DM).T
        ).astype(bf)
        wk_c = np.ascontiguousarray(
            np.asarray(Wk[hs], dtype=np.float32).reshape(EHC, DM).T
        ).astype(bf)
        wv_c = np.ascontiguousarray(
            np.asarray(Wv[hs], dtype=np.float32).reshape(EHC, DM).T
        ).astype(bf)
        wo_c = np.ascontiguousarray(
            np.asarray(Wout[EHC * r : EHC * (r + 1), :], dtype=np.float32).T
        ).astype(bf)
        in_maps.append(
            {
                "xqt": xt[g][0],
                "xkt": xt[g][1],
                "xvt": xt[g][2],
                "wqt": wq_c,
                "wkt": wk_c,
                "wvt": wv_c,
                "wot": wo_c,
            }
        )
    return in_maps


def kernel(x_query, x_key, x_value, Wq, Wk, Wv, Wout, _trace=False):
    global _cached_nc
    if _cached_nc is None:
        _cached_nc = build_nc()
    nc = _cached_nc

    in_maps = _prep_inputs(x_query, x_key, x_value, Wq, Wk, Wv, Wout)
    res = run_bass_kernel_spmd(nc, in_maps, list(range(NCORES)), trace=_trace)
    kernel.last_result = res

    out = np.empty((B, S, DM), dtype=np.float32)
    for c in range(NCORES):
        g, r = c // 4, c % 4
        out[g, :, EHC * r : EHC * (r + 1)] = res.results[c]["outt"].T
    return out


# revision 26
# speedup vs baseline: 1.2400x; 1.2400x over previous
"""Multi-head attention (B=2, S=2048, D=1024, H=16, d_head=64) on 8 TRN2 cores.

Sharding: 2-way data parallel over batch x 4-way tensor parallel over heads.
Core c: batch g = c//4, heads [4r, 4r+4) with r = c%4. Each core projects
Q/K/V for its 4 heads from its batch's (pre-transposed) activations, runs
attention per head in a transposed layout (scores^T with keys on partitions),
then AllGathers the per-core head outputs within each 4-core batch group and
computes a 256-row slice of the Wout projection (column parallel). The host
concatenates the per-core output slices.

Layout notes:
  - All matmul operands bf16; accumulation f32 in PSUM.
  - scores^T tiles [128 k, 2048 q] are written to PSUM as bf16 (2 banks,
    non-accumulating) so one ACT exp instruction covers a full k-tile.
  - softmax denominators ride as a 65th "ones" column of V in the PV matmul;
    normalization multiplies by the PE-broadcast reciprocal row.
"""

import os
import sys

import numpy as np

for _p in ("/opt/trn_rl_repo",):
    if _p not in sys.path and os.path.isdir(_p):
        sys.path.append(_p)

import ml_dtypes

import concourse.bacc as bacc
import concourse.bass_utils as _bu
import concourse.mybir as mybir
from concourse.bass_utils import run_bass_kernel_spmd
from concourse.tile import TileContext

# Let walrus dedup LDWEIGHTS for consecutive matmuls that share a stationary
# operand; without it every matmul reloads its weights and the reload gaps
# keep the PE clock throttled.
if not getattr(_bu, "_ldw_opt_patched", False):
    _orig_run_command = _bu.run_command

    def _run_command_ldw(cmd, *a, **kw):
        cmd = [
            c  # ldw-opt=true fails walrus codegen on this BIR; keep default
            if isinstance(c, str)
            else c
            for c in cmd
        ]
        return _orig_run_command(cmd, *a, **kw)

    _bu.run_command = _run_command_ldw
    _bu._ldw_opt_patched = True

P = 128
B, S, DM = 2, 2048, 1024
NH_TOT, EH = 16, 64  # total heads, head dim
NCORES = 8
GROUPS = 2  # batch groups of 4 cores
NH = 4  # heads per core
EHC = NH * EH  # 256: head-concat width per core
NDT = DM // P  # 8 d-tiles
NKT = S // P  # 16 key tiles
QC = 512  # q chunk
NQC = S // QC  # 4
VW = EH + 1  # V width incl. ones column

BF = mybir.dt.bfloat16
F32 = mybir.dt.float32
F32R = mybir.dt.float32r

_cached_nc = None


def build_nc():
    nc = bacc.Bacc("TRN2", target_bir_lowering=False, debug=False, num_devices=NCORES)

    xqt = nc.declare_dram_parameter("xqt", [DM, S], BF, isOutput=False)
    xkt = nc.declare_dram_parameter("xkt", [DM, S], BF, isOutput=False)
    xvt = nc.declare_dram_parameter("xvt", [DM, S], BF, isOutput=False)
    wqt = nc.declare_dram_parameter("wqt", [DM, EHC], BF, isOutput=False)
    wkt = nc.declare_dram_parameter("wkt", [DM, EHC], BF, isOutput=False)
    wvt = nc.declare_dram_parameter("wvt", [DM, EHC], BF, isOutput=False)
    wot = nc.declare_dram_parameter("wot", [DM, EHC], BF, isOutput=False)
    outt = nc.declare_dram_parameter("outt", [EHC, S], F32, isOutput=True)

    with TileContext(nc) as tc:
        with (
            tc.tile_pool(name="persist", bufs=1) as persist,
            tc.tile_pool(name="dram", bufs=1, space="DRAM") as dram,
        ):
            # --- persistent SBUF ---
            wq_sb = persist.tile([P, NDT, EHC], BF)
            wk_sb = persist.tile([P, NDT, EHC], BF)
            wv_sb = persist.tile([P, NDT, EHC], BF)
            wo_sb = persist.tile([P, NDT, EHC], BF)
            for wsb, wpar in ((wq_sb, wqt), (wk_sb, wkt), (wv_sb, wvt), (wo_sb, wot)):
                nc.sync.dma_start(wsb[:], wpar.rearrange("(dt p) e -> p dt e", p=P))

            qt_sb = [persist.tile([P, S], BF, name=f"qt{et}") for et in range(2)]
            kt_sb = [persist.tile([P, S], BF, name=f"kt{et}") for et in range(2)]
            v_sb = persist.tile([P, NKT * NH * VW + P - VW], BF)
            heads_sb = [persist.tile([EH, S], BF, name=f"hd{h}") for h in range(NH)]
            xv_sb = [persist.tile([P, S], BF, name=f"xv{dt}") for dt in range(NDT)]
            nc.gpsimd.memset(v_sb[:], 1.0)  # ones column; V data overwrites cols 0:64

            for dt in range(NDT):
                nc.sync.dma_start(xv_sb[dt][:], xvt[dt * P : (dt + 1) * P, :])

            heads_loc = [
                dram.tile([EHC, 1024], BF, name=f"hloc{qh}") for qh in range(2)
            ]
            heads_all = [
                dram.tile([4 * EHC, 1024], BF, name=f"hall_d{qh}") for qh in range(2)
            ]

            def emit_allgather(qh):
                nc.gpsimd.collective_compute(
                    "AllGather",
                    mybir.AluOpType.bypass,
                    replica_groups=[[0, 1, 2, 3], [4, 5, 6, 7]],
                    ins=[heads_loc[qh].opt()],
                    outs=[heads_all[qh].opt()],
                )

            # --- Q/K projections: Q^T/K^T = W^T.T @ x^T, e on partitions ---
            with (
                tc.tile_pool(name="xin", bufs=3) as xin,
                tc.tile_pool(name="projp", bufs=1, space="PSUM") as projp,
            ):
                for xpar, wsb, dst in ((xqt, wq_sb, qt_sb), (xkt, wk_sb, kt_sb)):
                    ps = [
                        [
                            projp.tile([P, QC], F32, name=f"pp{et}_{qc}")
                            for qc in range(NQC)
                        ]
                        for et in range(2)
                    ]
                    for dt in range(NDT):
                        xt = xin.tile([P, S], BF, name="xt", tag="xt")
                        nc.sync.dma_start(xt[:], xpar[dt * P : (dt + 1) * P, :])
                        for et in range(2):
                            for qc in range(NQC):
                                nc.tensor.matmul(
                                    ps[et][qc][:],
                                    wsb[:, dt, et * P : (et + 1) * P],
                                    xt[:, qc * QC : (qc + 1) * QC],
                                    start=(dt == 0),
                                    stop=(dt == NDT - 1),
                                )
                    for et in range(2):
                        for qc in range(NQC):
                            nc.vector.tensor_copy(
                                dst[et][:, qc * QC : (qc + 1) * QC], ps[et][qc][:]
                            )

            # --- V projection (token-major): V = x^T.T @ Wv^T, tok on partitions ---
            with tc.tile_pool(name="vp", bufs=2, space="PSUM") as vp:
                for tt in range(NKT):
                    psv = vp.tile([P, EHC], F32, name="psv")
                    for dt in range(NDT):
                        nc.tensor.matmul(
                            psv[:],
                            xv_sb[dt][:, tt * P : (tt + 1) * P],
                            wv_sb[:, dt, :],
                            start=(dt == 0),
                            stop=(dt == NDT - 1),
                        )
                    nc.vector.tensor_copy(
                        v_sb[:, tt * NH * VW : (tt + 1) * NH * VW].rearrange(
                            "p (h w) -> p h w", w=VW
                        )[:, :, 0:EH],
                        psv[:].rearrange("p (h e) -> p h e", e=EH),
                    )

            # --- attention: head pairs (e-tiles), row-tiled scores ---
            # The two heads of an e-tile compute scores concurrently via PE
            # row tiling (head A on array rows 0-63, head B on 64-127), and
            # PV uses a [128,128] stationary (V | ones | junk) so every
            # matmul drives the full array. q is processed in halves of 1024
            # so the 8 PSUM banks cover pair scores (4) + 4 PV accumulators.
            with (
                tc.tile_pool(name="scorep", bufs=1, space="PSUM") as scorep,
                tc.tile_pool(name="pvp", bufs=1, space="PSUM") as pvp,
                tc.tile_pool(name="exps", bufs=3) as expp,
                tc.tile_pool(name="normp", bufs=2) as normp,
            ):

                def normalize(h, qh, col0, pvt):
                    # heads[h][:, col0:col0+512] = pv[0:64] * bcast(1/pv[64]).
                    # The 1-lane denominator row is bounced through DRAM to
                    # spread it across 128 partitions for a fast reciprocal,
                    # then broadcast-read back across 64 partitions.
                    den = normp.tile([VW, QC], F32, name="den", tag="den")
                    nc.vector.tensor_copy(den[EH : EH + 1, :], pvt[EH : EH + 1, :])
                    den_d = dram.tile([QC], F32, name="den_d", tag="den_d", bufs=2)
                    nc.sync.dma_start(den_d[:], den[EH : EH + 1, :])
                    dsp = normp.tile([P, NQC], F32, name="dsp", tag="dsp")
                    nc.sync.dma_start(dsp[:], den_d[:].rearrange("(p f) -> p f", p=P))
                    rsp = normp.tile([P, NQC], F32, name="rsp", tag="rsp")
                    nc.vector.reciprocal(rsp[:], dsp[:])
                    rcp_d = dram.tile([QC], F32, name="rcp_d", tag="rcp_d", bufs=2)
                    nc.sync.dma_start(rcp_d[:].rearrange("(p f) -> p f", p=P), rsp[:])
                    bc = normp.tile([EH, QC], F32, name="bc", tag="bc")
                    nc.sync.dma_start(bc[:], rcp_d[None, :].to_broadcast([EH, QC]))
                    nc.vector.tensor_mul(
                        heads_sb[h][:, col0 : col0 + QC],
                        pvt[0:EH, :],
                        bc[:],
                    )
                    nc.sync.dma_start(
                        heads_loc[qh][
                            h * EH : (h + 1) * EH, col0 - qh * 1024 : col0 - qh * 1024 + QC
                        ],
                        heads_sb[h][:, col0 : col0 + QC],
                    )

                def voff(kt, h):
                    return (kt * NH + h) * VW

                # q-half outer so that after two sweeps every head has its
                # first 1024 q columns finished and the (expensive, mostly
                # fixed-cost) AllGather for that half launches early and
                # hides under the remaining sweeps.
                pending = []  # deferred (h, qh, col0, pv_tile) normalizations
                for qh in range(2):
                    q0 = qh * 1024
                    for ep in range(2):
                        hA, hB = 2 * ep, 2 * ep + 1
                        pv = [
                            [
                                pvp.tile(
                                    [P, QC], F32, name=f"pv{lh}{q2}", tag=f"pv{lh}{q2}"
                                )
                                for q2 in range(2)
                            ]
                            for lh in range(2)
                        ]
                        exring = [None] * NKT
                        for kt in range(NKT + 1):
                            if kt < NKT:
                                # scores + exp for kt (one tile ahead of PV so
                                # the PV matmuls overlap the next exp)
                                expair = expp.tile(
                                    [P, S], BF, name="expair", tag="expair"
                                )
                                exring[kt] = expair
                                spair = scorep.tile(
                                    [P, S], F32, name="spair", tag="spair"
                                )
                                for sub in range(2):
                                    qs = q0 + sub * QC
                                    nc.tensor.matmul(
                                        spair[:, sub * QC : (sub + 1) * QC],
                                        kt_sb[ep][0:EH, kt * P : (kt + 1) * P],
                                        qt_sb[ep][0:EH, qs : qs + QC],
                                        start=True,
                                        stop=True,
                                    )
                                    nc.tensor.matmul(
                                        spair[
                                            :, 1024 + sub * QC : 1024 + (sub + 1) * QC
                                        ],
                                        kt_sb[ep][EH:P, kt * P : (kt + 1) * P],
                                        qt_sb[ep][EH:P, qs : qs + QC],
                                        start=True,
                                        stop=True,
                                    )
                                nc.scalar.activation(
                                    expair[:],
                                    spair[:],
                                    mybir.ActivationFunctionType.Exp,
                                    scale=float(1.0 / np.sqrt(EH)),
                                )
                            if kt >= 1:
                                pkt = kt - 1
                                for lh in range(2):
                                    h = hA if lh == 0 else hB
                                    for q2 in range(2):
                                        nc.tensor.matmul(
                                            pv[lh][q2][:],
                                            v_sb[:, voff(pkt, h) : voff(pkt, h) + P],
                                            exring[pkt][
                                                :,
                                                lh * 1024 + q2 * QC : lh * 1024
                                                + (q2 + 1) * QC,
                                            ],
                                            start=(pkt == 0),
                                            stop=(pkt == NKT - 1),
                                            skip_group_check=True,
                                        )
                            if kt == 3 and pending:
                                flush_qh, flush_ep = pending[0][1], pending[0][2]
                                for ph, _, _, pcol0, ppv in pending:
                                    normalize(ph, flush_qh, pcol0, ppv)
                                pending = []
                                if flush_qh == 0 and flush_ep == 1:
                                    # all four heads' first q-half is done
                                    emit_allgather(0)
                        for lh in range(2):
                            h = hA if lh == 0 else hB
                            for q2 in range(2):
                                pending.append((h, qh, ep, q0 + q2 * QC, pv[lh][q2]))
                flush_qh = pending[0][1]
                for ph, _, _, pcol0, ppv in pending:
                    normalize(ph, flush_qh, pcol0, ppv)
                emit_allgather(1)

            # --- Wout (column-parallel slice): out^T = Wout_slice^T.T @ heads^T ---
            with (
                tc.tile_pool(name="hall", bufs=1) as hallp,
                tc.tile_pool(name="wop", bufs=2, space="PSUM") as wop,
                tc.tile_pool(name="outp", bufs=1) as outp,
            ):
                hall = [
                    [hallp.tile([P, 1024], BF, name=f"hall{qh}_{dt}") for dt in range(NDT)]
                    for qh in range(2)
                ]
                for qh in range(2):
                    for dt in range(NDT):
                        nc.sync.dma_start(
                            hall[qh][dt][:], heads_all[qh][dt * P : (dt + 1) * P, :]
                        )
                out_sb = [outp.tile([P, S], F32, name=f"ot{ot}") for ot in range(2)]
                for qh in range(2):
                    for ot in range(2):
                        for q2 in range(2):
                            pso = wop.tile([P, QC], F32, name="pso", tag="pso")
                            for dt in range(NDT):
                                nc.tensor.matmul(
                                    pso[:],
                                    wo_sb[:, dt, ot * P : (ot + 1) * P],
                                    hall[qh][dt][:, q2 * QC : (q2 + 1) * QC],
                                    start=(dt == 0),
                                    stop=(dt == NDT - 1),
                                )
                            nc.vector.tensor_copy(
                                out_sb[ot][:, qh * 1024 + q2 * QC : qh * 1024 + (q2 + 1) * QC],
                                pso[:],
                            )
                for ot in range(2):
                    nc.sync.dma_start(outt[ot * P : (ot + 1) * P, :], out_sb[ot][:])

    nc.compile()
    return nc


def _prep_inputs(x_query, x_key, x_value, Wq, Wk, Wv, Wout):
    bf = ml_dtypes.bfloat16
    xt = {}
    for g in range(GROUPS):
        xt[g] = tuple(
            np.ascontiguousarray(np.asarray(x[g], dtype=np.float32).T).astype(bf)
            for x in (x_query, x_key, x_value)
        )
    in_maps = []
    for c in range(NCORES):
        g, r = c // 4, c % 4
        hs = slice(NH * r, NH * (r + 1))
        wq_c = np.ascontiguousarray(
            np.asarray(Wq[hs], dtype=np.float32).reshape(EHC, DM).T
        ).astype(bf)
        wk_c = np.ascontiguousarray(
            np.asarray(Wk[hs], dtype=np.float32).reshape(EHC, DM).T
        ).astype(bf)
        wv_c = np.ascontiguousarray(
            np.asarray(Wv[hs], dtype=np.float32).reshape(EHC, DM).T
        ).astype(bf)
        wo_c = np.ascontiguousarray(
            np.asarray(Wout[EHC * r : EHC * (r + 1), :], dtype=np.float32).T
        ).astype(bf)
        in_maps.append(
            {
                "xqt": xt[g][0],
                "xkt": xt[g][1],
                "xvt": xt[g][2],
                "wqt": wq_c,
                "wkt": wk_c,
                "wvt": wv_c,
                "wot": wo_c,
            }
        )
    return in_maps


def kernel(x_query, x_key, x_value, Wq, Wk, Wv, Wout, _trace=False):
    global _cached_nc
    if _cached_nc is None:
        _cached_nc = build_nc()
    nc = _cached_nc

    in_maps = _prep_inputs(x_query, x_key, x_value, Wq, Wk, Wv, Wout)
    res = run_bass_kernel_spmd(nc, in_maps, list(range(NCORES)), trace=_trace)
    kernel.last_result = res

    out = np.empty((B, S, DM), dtype=np.float32)
    for c in range(NCORES):
        g, r = c // 4, c % 4
        out[g, :, EHC * r : EHC * (r + 1)] = res.results[c]["outt"].T
    return out


# revision 27
# speedup vs baseline: 1.2550x; 1.0121x over previous
"""Multi-head attention (B=2, S=2048, D=1024, H=16, d_head=64) on 8 TRN2 cores.

Sharding: 2-way data parallel over batch x 4-way tensor parallel over heads.
Core c: batch g = c//4, heads [4r, 4r+4) with r = c%4. Each core projects
Q/K/V for its 4 heads from its batch's (pre-transposed) activations, runs
attention per head in a transposed layout (scores^T with keys on partitions),
then AllGathers the per-core head outputs within each 4-core batch group and
computes a 256-row slice of the Wout projection (column parallel). The host
concatenates the per-core output slices.

Layout notes:
  - All matmul operands bf16; accumulation f32 in PSUM.
  - scores^T tiles [128 k, 2048 q] are written to PSUM as bf16 (2 banks,
    non-accumulating) so one ACT exp instruction covers a full k-tile.
  - softmax denominators ride as a 65th "ones" column of V in the PV matmul;
    normalization multiplies by the PE-broadcast reciprocal row.
"""

import os
import sys

import numpy as np

for _p in ("/opt/trn_rl_repo",):
    if _p not in sys.path and os.path.isdir(_p):
        sys.path.append(_p)

import ml_dtypes

import concourse.bacc as bacc
import concourse.bass_utils as _bu
import concourse.mybir as mybir
from concourse.bass_utils import run_bass_kernel_spmd
from concourse.tile import TileContext

# Let walrus dedup LDWEIGHTS for consecutive matmuls that share a stationary
# operand; without it every matmul reloads its weights and the reload gaps
# keep the PE clock throttled.
if not getattr(_bu, "_ldw_opt_patched", False):
    _orig_run_command = _bu.run_command

    def _run_command_ldw(cmd, *a, **kw):
        cmd = [
            c  # ldw-opt=true fails walrus codegen on this BIR; keep default
            if isinstance(c, str)
            else c
            for c in cmd
        ]
        return _orig_run_command(cmd, *a, **kw)

    _bu.run_command = _run_command_ldw
    _bu._ldw_opt_patched = True

P = 128
B, S, DM = 2, 2048, 1024
NH_TOT, EH = 16, 64  # total heads, head dim
NCORES = 8
GROUPS = 2  # batch groups of 4 cores
NH = 4  # heads per core
EHC = NH * EH  # 256: head-concat width per core
NDT = DM // P  # 8 d-tiles
NKT = S // P  # 16 key tiles
QC = 512  # q chunk
NQC = S // QC  # 4
VW = EH + 1  # V width incl. ones column

BF = mybir.dt.bfloat16
F32 = mybir.dt.float32
F32R = mybir.dt.float32r

_cached_nc = None


def build_nc():
    nc = bacc.Bacc("TRN2", target_bir_lowering=False, debug=False, num_devices=NCORES)

    xqt = nc.declare_dram_parameter("xqt", [DM, S], BF, isOutput=False)
    xkt = nc.declare_dram_parameter("xkt", [DM, S], BF, isOutput=False)
    xvt = nc.declare_dram_parameter("xvt", [DM, S], BF, isOutput=False)
    wqt = nc.declare_dram_parameter("wqt", [DM, EHC], BF, isOutput=False)
    wkt = nc.declare_dram_parameter("wkt", [DM, EHC], BF, isOutput=False)
    wvt = nc.declare_dram_parameter("wvt", [DM, EHC], BF, isOutput=False)
    wot = nc.declare_dram_parameter("wot", [DM, EHC], BF, isOutput=False)
    outt = nc.declare_dram_parameter("outt", [EHC, S], F32, isOutput=True)

    with TileContext(nc) as tc:
        with (
            tc.tile_pool(name="persist", bufs=1) as persist,
            tc.tile_pool(name="dram", bufs=1, space="DRAM") as dram,
        ):
            # --- persistent SBUF ---
            wq_sb = persist.tile([P, NDT, EHC], BF)
            wk_sb = persist.tile([P, NDT, EHC], BF)
            wv_sb = persist.tile([P, NDT, EHC], BF)
            wo_sb = persist.tile([P, NDT, EHC], BF)
            for wsb, wpar in ((wq_sb, wqt), (wk_sb, wkt), (wv_sb, wvt), (wo_sb, wot)):
                nc.sync.dma_start(wsb[:], wpar.rearrange("(dt p) e -> p dt e", p=P))

            qt_sb = [persist.tile([P, S], BF, name=f"qt{et}") for et in range(2)]
            kt_sb = [persist.tile([P, S], BF, name=f"kt{et}") for et in range(2)]
            v_sb = persist.tile([P, NKT * NH * VW + P - VW], BF)
            heads_sb = [persist.tile([EH, S], BF, name=f"hd{h}") for h in range(NH)]
            xv_sb = [persist.tile([P, S], BF, name=f"xv{dt}") for dt in range(NDT)]
            nc.gpsimd.memset(v_sb[:], 1.0)  # ones column; V data overwrites cols 0:64

            for dt in range(NDT):
                nc.sync.dma_start(xv_sb[dt][:], xvt[dt * P : (dt + 1) * P, :])

            heads_loc = [
                dram.tile([EHC, 1024], BF, name=f"hloc{qh}") for qh in range(2)
            ]
            heads_all = [
                dram.tile([4 * EHC, 1024], BF, name=f"hall_d{qh}") for qh in range(2)
            ]

            def emit_allgather(qh):
                nc.gpsimd.collective_compute(
                    "AllGather",
                    mybir.AluOpType.bypass,
                    replica_groups=[[0, 1, 2, 3], [4, 5, 6, 7]],
                    ins=[heads_loc[qh].opt()],
                    outs=[heads_all[qh].opt()],
                )

            # --- Q/K projections: Q^T/K^T = W^T.T @ x^T, e on partitions ---
            with (
                tc.tile_pool(name="xin", bufs=3) as xin,
                tc.tile_pool(name="projp", bufs=1, space="PSUM") as projp,
            ):
                for xpar, wsb, dst in ((xqt, wq_sb, qt_sb), (xkt, wk_sb, kt_sb)):
                    ps = [
                        [
                            projp.tile([P, QC], F32, name=f"pp{et}_{qc}")
                            for qc in range(NQC)
                        ]
                        for et in range(2)
                    ]
                    for dt in range(NDT):
                        xt = xin.tile([P, S], BF, name="xt", tag="xt")
                        nc.sync.dma_start(xt[:], xpar[dt * P : (dt + 1) * P, :])
                        for et in range(2):
                            for qc in range(NQC):
                                nc.tensor.matmul(
                                    ps[et][qc][:],
                                    wsb[:, dt, et * P : (et + 1) * P],
                                    xt[:, qc * QC : (qc + 1) * QC],
                                    start=(dt == 0),
                                    stop=(dt == NDT - 1),
                                )
                    for et in range(2):
                        for qc in range(NQC):
                            nc.vector.tensor_copy(
                                dst[et][:, qc * QC : (qc + 1) * QC], ps[et][qc][:]
                            )

            # --- V projection (token-major): V = x^T.T @ Wv^T, tok on partitions ---
            with tc.tile_pool(name="vp", bufs=2, space="PSUM") as vp:
                for tt in range(NKT):
                    psv = vp.tile([P, EHC], F32, name="psv")
                    for dt in range(NDT):
                        nc.tensor.matmul(
                            psv[:],
                            xv_sb[dt][:, tt * P : (tt + 1) * P],
                            wv_sb[:, dt, :],
                            start=(dt == 0),
                            stop=(dt == NDT - 1),
                        )
                    nc.vector.tensor_copy(
                        v_sb[:, tt * NH * VW : (tt + 1) * NH * VW].rearrange(
                            "p (h w) -> p h w", w=VW
                        )[:, :, 0:EH],
                        psv[:].rearrange("p (h e) -> p h e", e=EH),
                    )

            # --- attention: head pairs (e-tiles), row-tiled scores ---
            # The two heads of an e-tile compute scores concurrently via PE
            # row tiling (head A on array rows 0-63, head B on 64-127), and
            # PV uses a [128,128] stationary (V | ones | junk) so every
            # matmul drives the full array. q is processed in halves of 1024
            # so the 8 PSUM banks cover pair scores (4) + 4 PV accumulators.
            with (
                tc.tile_pool(name="scorep", bufs=1, space="PSUM") as scorep,
                tc.tile_pool(name="pvp", bufs=1, space="PSUM") as pvp,
                tc.tile_pool(name="exps", bufs=3) as expp,
                tc.tile_pool(name="normp", bufs=2) as normp,
            ):

                def normalize(h, qh, col0, pvt):
                    # heads[h][:, col0:col0+512] = pv[0:64] * bcast(1/pv[64]).
                    # The 1-lane denominator row is bounced through DRAM to
                    # spread it across 128 partitions for a fast reciprocal,
                    # then broadcast-read back across 64 partitions.
                    den = normp.tile([VW, QC], F32, name="den", tag="den")
                    nc.vector.tensor_copy(den[EH : EH + 1, :], pvt[EH : EH + 1, :])
                    den_d = dram.tile([QC], F32, name="den_d", tag="den_d", bufs=2)
                    nc.sync.dma_start(den_d[:], den[EH : EH + 1, :])
                    dsp = normp.tile([P, NQC], F32, name="dsp", tag="dsp")
                    nc.sync.dma_start(dsp[:], den_d[:].rearrange("(p f) -> p f", p=P))
                    rsp = normp.tile([P, NQC], F32, name="rsp", tag="rsp")
                    nc.vector.reciprocal(rsp[:], dsp[:])
                    rcp_d = dram.tile([QC], F32, name="rcp_d", tag="rcp_d", bufs=2)
                    nc.sync.dma_start(rcp_d[:].rearrange("(p f) -> p f", p=P), rsp[:])
                    bc = normp.tile([EH, QC], F32, name="bc", tag="bc")
                    nc.sync.dma_start(bc[:], rcp_d[None, :].to_broadcast([EH, QC]))
                    nc.vector.tensor_mul(
                        heads_sb[h][:, col0 : col0 + QC],
                        pvt[0:EH, :],
                        bc[:],
                    )
                    nc.sync.dma_start(
                        heads_loc[qh][
                            h * EH : (h + 1) * EH, col0 - qh * 1024 : col0 - qh * 1024 + QC
                        ],
                        heads_sb[h][:, col0 : col0 + QC],
                    )

                def voff(kt, h):
                    return (kt * NH + h) * VW

                # q-half outer so that after two sweeps every head has its
                # first 1024 q columns finished and the (expensive, mostly
                # fixed-cost) AllGather for that half launches early and
                # hides under the remaining sweeps.
                pending = []  # deferred (h, qh, col0, pv_tile) normalizations
                for qh in range(2):
                    q0 = qh * 1024
                    for ep in range(2):
                        hA, hB = 2 * ep, 2 * ep + 1
                        pv = [
                            [
                                pvp.tile(
                                    [P, QC], F32, name=f"pv{lh}{q2}", tag=f"pv{lh}{q2}"
                                )
                                for q2 in range(2)
                            ]
                            for lh in range(2)
                        ]
                        exring = [None] * NKT
                        for kt in range(NKT + 1):
                            if kt < NKT:
                                # scores + exp for kt (one tile ahead of PV so
                                # the PV matmuls overlap the next exp); per-head
                                # score tiles so head A's next scores can start
                                # while head B's exp still runs
                                exa = expp.tile([P, 1024], BF, name="exa", tag="exa")
                                exb = expp.tile([P, 1024], BF, name="exb", tag="exb")
                                exring[kt] = (exa, exb)
                                for lh, ext in ((0, exa), (1, exb)):
                                    po = lh * EH
                                    s_t = scorep.tile(
                                        [P, 1024], F32, name=f"s{lh}", tag=f"s{lh}"
                                    )
                                    for sub in range(2):
                                        qs = q0 + sub * QC
                                        nc.tensor.matmul(
                                            s_t[:, sub * QC : (sub + 1) * QC],
                                            kt_sb[ep][
                                                po : po + EH, kt * P : (kt + 1) * P
                                            ],
                                            qt_sb[ep][po : po + EH, qs : qs + QC],
                                            start=True,
                                            stop=True,
                                        )
                                    nc.scalar.activation(
                                        ext[:],
                                        s_t[:],
                                        mybir.ActivationFunctionType.Exp,
                                        scale=float(1.0 / np.sqrt(EH)),
                                    )
                            if kt >= 1:
                                pkt = kt - 1
                                for lh in range(2):
                                    h = hA if lh == 0 else hB
                                    for q2 in range(2):
                                        nc.tensor.matmul(
                                            pv[lh][q2][:],
                                            v_sb[:, voff(pkt, h) : voff(pkt, h) + P],
                                            exring[pkt][lh][
                                                :, q2 * QC : (q2 + 1) * QC
                                            ],
                                            start=(pkt == 0),
                                            stop=(pkt == NKT - 1),
                                            skip_group_check=True,
                                        )
                            if kt == 3 and pending:
                                flush_qh, flush_ep = pending[0][1], pending[0][2]
                                for ph, _, _, pcol0, ppv in pending:
                                    normalize(ph, flush_qh, pcol0, ppv)
                                pending = []
                                if flush_qh == 0 and flush_ep == 1:
                                    # all four heads' first q-half is done
                                    emit_allgather(0)
                        for lh in range(2):
                            h = hA if lh == 0 else hB
                            for q2 in range(2):
                                pending.append((h, qh, ep, q0 + q2 * QC, pv[lh][q2]))
                flush_qh = pending[0][1]
                for ph, _, _, pcol0, ppv in pending:
                    normalize(ph, flush_qh, pcol0, ppv)
                emit_allgather(1)

            # --- Wout (column-parallel slice): out^T = Wout_slice^T.T @ heads^T ---
            with (
                tc.tile_pool(name="hall", bufs=1) as hallp,
                tc.tile_pool(name="wop", bufs=2, space="PSUM") as wop,
                tc.tile_pool(name="outp", bufs=1) as outp,
            ):
                hall = [
                    [hallp.tile([P, 1024], BF, name=f"hall{qh}_{dt}") for dt in range(NDT)]
                    for qh in range(2)
                ]
                for qh in range(2):
                    for dt in range(NDT):
                        nc.sync.dma_start(
                            hall[qh][dt][:], heads_all[qh][dt * P : (dt + 1) * P, :]
                        )
                out_sb = [outp.tile([P, S], F32, name=f"ot{ot}") for ot in range(2)]
                for qh in range(2):
                    for ot in range(2):
                        for q2 in range(2):
                            pso = wop.tile([P, QC], F32, name="pso", tag="pso")
                            for dt in range(NDT):
                                nc.tensor.matmul(
                                    pso[:],
                                    wo_sb[:, dt, ot * P : (ot + 1) * P],
                                    hall[qh][dt][:, q2 * QC : (q2 + 1) * QC],
                                    start=(dt == 0),
                                    stop=(dt == NDT - 1),
                                )
                            nc.vector.tensor_copy(
                                out_sb[ot][:, qh * 1024 + q2 * QC : qh * 1024 + (q2 + 1) * QC],
                                pso[:],
                            )
                for ot in range(2):
                    nc.sync.dma_start(outt[ot * P : (ot + 1) * P, :], out_sb[ot][:])

    nc.compile()
    return nc


def _prep_inputs(x_query, x_key, x_value, Wq, Wk, Wv, Wout):
    bf = ml_dtypes.bfloat16
    xt = {}
    for g in range(GROUPS):
        xt[g] = tuple(
            np.ascontiguousarray(np.asarray(x[g], dtype=np.float32).T).astype(bf)
            for x in (x_query, x_key, x_value)
        )
    in_maps = []
    for c in range(NCORES):
        g, r = c // 4, c % 4
        hs = slice(NH * r, NH * (r + 1))
        wq_c = np.ascontiguousarray(
            np.asarray(Wq[hs], dtype=np.float32).reshape(EHC, DM).T
        ).astype(bf)
        wk_c = np.ascontiguousarray(
            np.asarray(Wk[hs], dtype=np.float32).reshape(EHC, DM).T
        ).astype(bf)
        wv_c = np.ascontiguousarray(
            np.asarray(Wv[hs], dtype=np.float32).reshape(EHC, DM).T
        ).astype(bf)
        wo_c = np.ascontiguousarray(
            np.asarray(Wout[EHC * r : EHC * (r + 1), :], dtype=np.float32).T
        ).astype(bf)
        in_maps.append(
            {
                "xqt": xt[g][0],
                "xkt": xt[g][1],
                "xvt": xt[g][2],
                "wqt": wq_c,
                "wkt": wk_c,
                "wvt": wv_c,
                "wot": wo_c,
            }
        )
    return in_maps


def kernel(x_query, x_key, x_value, Wq, Wk, Wv, Wout, _trace=False):
    global _cached_nc
    if _cached_nc is None:
        _cached_nc = build_nc()
    nc = _cached_nc

    in_maps = _prep_inputs(x_query, x_key, x_value, Wq, Wk, Wv, Wout)
    res = run_bass_kernel_spmd(nc, in_maps, list(range(NCORES)), trace=_trace)
    kernel.last_result = res

    out = np.empty((B, S, DM), dtype=np.float32)
    for c in range(NCORES):
        g, r = c // 4, c % 4
        out[g, :, EHC * r : EHC * (r + 1)] = res.results[c]["outt"].T
    return out
